# revision 1
# baseline (speedup 1.0000x reference)
"""GNN attention message-passing kernel for TRN2, 8-core SPMD.

Math (exact up to fp32 rounding; softmax shift-invariance removes the dst-side
attention term and constant biases):
    alpha_e = softmax over incoming edges of dst_e of  b[src_e]
    b[n]    = h[n] @ v,  v = W_coef @ W_red[128:, 0]
    agg[d]  = sum_e alpha_e h[src_e]
    out[d]  = l2norm([h[d] @ W_node + b_node | agg[d] @ W_neigh + b_neigh])

Device:
    x[n] = exp(b[n]);  T[n] = [x[n]*(h[n] @ W_neigh) | x[n]]   (129 f32 / row)
    numer|denom[d] = segment-sum of T[src_e] over incoming edges
    neigh[d] = numer/denom + b_neigh

Sharding: core = (dst_quarter, src_half); src half split at N/2 keeps
dma_gather indices in int16 range.  Pairwise ReduceScatter merges the two
src-halves of each quarter before the finalize pass.
"""

import numpy as np

import concourse.bass as bass
import concourse.bacc as bacc
import concourse.mybir as mybir
import concourse.tile as tile
from concourse.masks import make_identity
from concourse.tile_rust import add_dep_helper

F32 = mybir.dt.float32
I16 = mybir.dt.int16
I32 = mybir.dt.int32
EPS = 1e-12
D = 128
TSTRIDE = 192  # table row stride in f32 elems (768B, 256B multiple)
AF = mybir.ActivationFunctionType
ALU = mybir.AluOpType


# ---------------------------------------------------------------- host prep
def prep(src, dst, N, sslot=896, verbose=False):
    NC = 8
    Q = N // 4
    HALF = N // 2
    SH = HALF // 4
    FINROWS = ((Q // 2 + 127) // 128 + 1) * 128
    PBUF = 2 * FINROWS

    quarter = dst // Q
    half = (src >= HALF).astype(np.int64)
    core = quarter * 2 + half
    order = np.lexsort((dst, core))
    src_s, dst_s, core_s = src[order], dst[order], core[order]
    bounds = np.searchsorted(core_s, np.arange(NC + 1))

    while True:
        ok = True
        per_core = []
        for c in range(NC):
            lo, hi = bounds[c], bounds[c + 1]
            cs = src_s[lo:hi] - (c & 1) * HALF
            cd = dst_s[lo:hi] - (c >> 1) * Q
            grp = np.flatnonzero(np.r_[True, cd[1:] != cd[:-1]])
            grp = np.r_[grp, len(cd)]
            strips = []
            gi = 0
            while gi < len(grp) - 1:
                e0 = grp[gi]
                base = cd[e0]
                gj = gi
                while gj + 1 < len(grp):
                    ge = grp[gj + 1]
                    if ge - e0 <= sslot and (cd[ge - 1] - base) < 128:
                        gj += 1
                    else:
                        break
                e1 = grp[gj]
                if e1 == e0:
                    ok = False
                    break
                strips.append((int(base), int(e0), int(e1)))
                gi = gj
            if not ok:
                break
            per_core.append((cs, cd, strips))
        if ok:
            break
        sslot -= 128
        assert sslot >= 256, "could not build uniform strips"

    nstrip = max(len(p[2]) for p in per_core)
    nch = sslot // 128
    nslot = nstrip * sslot
    padbase = PBUF - 128

    idx_all, dstm_all, base_all = [], [], []
    for c in range(NC):
        cs, cd, strips = per_core[c]
        idx = np.zeros(nslot, np.int16)
        dstm = np.full(nslot, -1.0, np.float32)
        bases = np.full(nstrip, padbase, np.int32)
        for k, (b, e0, e1) in enumerate(strips):
            n = e1 - e0
            idx[k * sslot:k * sslot + n] = cs[e0:e1]
            dstm[k * sslot:k * sslot + n] = (cd[e0:e1] - b).astype(np.float32)
            bases[k] = b
        idxw = np.tile(np.ascontiguousarray(idx.reshape(-1, 16).T), (8, 1))
        dstmw = np.ascontiguousarray(dstm.reshape(-1, 128).T)
        idx_all.append(idxw)
        dstm_all.append(dstmw)
        base_all.append(np.ascontiguousarray(bases.reshape(1, -1)))

    cfg = dict(N=N, NC=NC, Q=Q, HALF=HALF, SH=SH, FINROWS=FINROWS, PBUF=PBUF,
               SSLOT=sslot, NCH=nch, NSTRIP=nstrip, NSLOT=nslot,
               NCHTOT=nslot // 128, PADBASE=padbase)
    if verbose:
        used = [len(p[2]) for p in per_core]
        print(f"prep: sslot={sslot} nstrip={nstrip} used={used} "
              f"slots/core={nslot}")
    return cfg, idx_all, dstm_all, base_all


def host_inputs(cfg, h, W_coef, W_red, W_node, b_node, W_neigh, b_neigh,
                idx_all, dstm_all, base_all):
    Q, HALF, SH, FIN = cfg["Q"], cfg["HALF"], cfg["SH"], cfg["FINROWS"]
    iota2 = np.ascontiguousarray(
        np.tile(np.arange(128, dtype=np.float32), (128, 1)))
    maps = []
    for c in range(8):
        q, hf = c >> 1, c & 1
        s0 = hf * HALF + q * SH
        f0 = q * Q + hf * FIN
        f1 = min(f0 + FIN, (q + 1) * Q)
        hfin = np.zeros((FIN, D), np.float32)
        hfin[:f1 - f0] = h[f0:f1]
        maps.append({
            "h1": np.ascontiguousarray(h[s0:s0 + SH]),
            "hfin": hfin,
            "Wcoef": W_coef,
            "w2": np.ascontiguousarray(W_red[D:2 * D, 0:1]),
            "Wnode": W_node,
            "bnode": np.ascontiguousarray(b_node.reshape(1, D)),
            "Wneigh": W_neigh,
            "bneigh": np.ascontiguousarray(b_neigh.reshape(1, D)),
            "idxw": idx_all[c],
            "dstm": dstm_all[c],
            "bases": base_all[c],
            "iota2": iota2,
        })
    return maps


def assemble(cfg, results):
    N, Q, FIN = cfg["N"], cfg["Q"], cfg["FINROWS"]
    out = np.zeros((N, 2 * D), np.float32)
    for q in range(4):
        out[q * Q:q * Q + FIN] = results[2 * q]["out"]
        out[q * Q + FIN:(q + 1) * Q] = results[2 * q + 1]["out"][:Q - FIN]
    return out


# ---------------------------------------------------------------- device
def bcast_mid(ap2d, reps):
    """[P, C] -> [P, C, reps] with inner step 0 (free-dim broadcast)."""
    a = ap2d
    return bass.AP(a.tensor, a.offset, [a.ap[0], a.ap[1], [0, reps]])


def tile_mid(ap2d, reps):
    """[P, C] -> [P, reps, C] repeating the row block (middle step 0)."""
    a = ap2d
    return bass.AP(a.tensor, a.offset, [a.ap[0], [0, reps], a.ap[1]])


def build(cfg, newton=1, dma_queues=2, scratch=32768):
    Q, HALF, SH = cfg["Q"], cfg["HALF"], cfg["SH"]
    FIN, PBUF = cfg["FINROWS"], cfg["PBUF"]
    SSLOT, NCH, NSTRIP, NSLOT = cfg["SSLOT"], cfg["NCH"], cfg["NSTRIP"], cfg["NSLOT"]
    NCHTOT = cfg["NCHTOT"]

    nc = bacc.Bacc("TRN2", target_bir_lowering=False, debug=False,
                   num_devices=8, dynamic_dma_scratch_size=scratch,
                   num_swdge_queues=dma_queues)

    h1_d = nc.dram_tensor("h1", [SH, D], F32, kind="ExternalInput").ap()
    hfin_d = nc.dram_tensor("hfin", [FIN, D], F32, kind="ExternalInput").ap()
    wcoef_d = nc.dram_tensor("Wcoef", [D, D], F32, kind="ExternalInput").ap()
    w2_d = nc.dram_tensor("w2", [D, 1], F32, kind="ExternalInput").ap()
    wnode_d = nc.dram_tensor("Wnode", [D, D], F32, kind="ExternalInput").ap()
    bnode_d = nc.dram_tensor("bnode", [1, D], F32, kind="ExternalInput").ap()
    wneigh_d = nc.dram_tensor("Wneigh", [D, D], F32, kind="ExternalInput").ap()
    bneigh_d = nc.dram_tensor("bneigh", [1, D], F32, kind="ExternalInput").ap()
    idxw_d = nc.dram_tensor("idxw", [128, NSLOT // 16], I16, kind="ExternalInput").ap()
    dstm_d = nc.dram_tensor("dstm", [128, NCHTOT], F32, kind="ExternalInput").ap()
    bases_d = nc.dram_tensor("bases", [1, NSTRIP], I32, kind="ExternalInput").ap()
    iota_d = nc.dram_tensor("iota2", [128, 128], F32, kind="ExternalInput").ap()
    out_d = nc.dram_tensor("out", [FIN, 2 * D], F32, kind="ExternalOutput").ap()

    tsh_d = nc.dram_tensor("tsh", [SH, TSTRIDE], F32).ap()
    thalf_d = nc.dram_tensor("thalf", [HALF, TSTRIDE], F32).ap()
    part_d = nc.dram_tensor("part", [PBUF, D + 1], F32).ap()
    rsout_d = nc.dram_tensor("rsout", [FIN, D + 1], F32).ap()

    with tile.TileContext(nc) as tc:
        with tc.tile_pool(name="const", bufs=1) as cpool, \
             tc.tile_pool(name="s1", bufs=3) as s1pool, \
             tc.tile_pool(name="gath", bufs=4) as gpool, \
             tc.tile_pool(name="stp", bufs=4) as stpool, \
             tc.tile_pool(name="okp", bufs=4) as okpool, \
             tc.tile_pool(name="fin", bufs=3) as fpool, \
             tc.tile_pool(name="ps", bufs=4, space="PSUM") as pspool, \
             tc.tile_pool(name="ps2", bufs=2, space="PSUM") as ps2pool:

            ident = cpool.tile([128, 128], F32)
            make_identity(nc, ident[:])
            iota2 = cpool.tile([128, 128], F32)
            nc.sync.dma_start(iota2[:], iota_d[:])

            # Wcat = [W_neigh | v]
            wcat = cpool.tile([128, D + 1], F32)
            nc.sync.dma_start(wcat[:, 0:D], wneigh_d[:])
            wc = s1pool.tile([128, 128], F32, tag="wc")
            nc.sync.dma_start(wc[:], wcoef_d[:])
            w2t = s1pool.tile([128, 1], F32, tag="w2")
            nc.sync.dma_start(w2t[:], w2_d[:])
            pst = ps2pool.tile([128, 128], F32, tag="tr", space="PSUM")
            nc.tensor.transpose(out=pst[:], in_=wc[:], identity=ident[:])
            wcT = s1pool.tile([128, 128], F32, tag="wcT")
            nc.vector.tensor_copy(wcT[:], pst[:])
            psv = ps2pool.tile([128, 1], F32, tag="v", space="PSUM")
            nc.tensor.matmul(psv[:], lhsT=wcT[:], rhs=w2t[:], start=True, stop=True)
            nc.vector.tensor_copy(wcat[:, D:D + 1], psv[:])

            # ---- stage 1: T shard
            tsh_writes = []
            nchunk1 = (SH + 127) // 128
            for i in range(nchunk1):
                r0 = i * 128
                nr = min(128, SH - r0)
                hch = s1pool.tile([128, 128], F32, tag="hch")
                nc.sync.dma_start(hch[:nr, :], h1_d[r0:r0 + nr, :])
                pstr = ps2pool.tile([128, 128], F32, tag="tr", space="PSUM")
                nc.tensor.transpose(out=pstr[:, :nr], in_=hch[:nr, :],
                                    identity=ident[:])
                hT = s1pool.tile([128, 128], F32, tag="hT")
                nc.vector.tensor_copy(hT[:, :nr], pstr[:, :nr])
                ps1 = ps2pool.tile([128, D + 1], F32, tag="s1", space="PSUM")
                nc.tensor.matmul(ps1[:nr, :], lhsT=hT[:, :nr], rhs=wcat[:],
                                 start=True, stop=True)
                xcol = s1pool.tile([128, 1], F32, tag="xc")
                nc.scalar.activation(xcol[:nr, :], ps1[:nr, D:D + 1], AF.Exp)
                tt = s1pool.tile([128, D + 1], F32, tag="tt")
                nc.vector.tensor_scalar(out=tt[:nr, 0:D], in0=ps1[:nr, 0:D],
                                        scalar1=xcol[:nr, :], scalar2=None,
                                        op0=ALU.mult)
                nc.vector.tensor_copy(tt[:nr, D:D + 1], xcol[:nr, :])
                w = nc.sync.dma_start(tsh_d[r0:r0 + nr, 0:D + 1], tt[:nr, :])
                tsh_writes.append(w)

            # ---- allgather half-table
            ag = nc.gpsimd.collective_compute(
                "AllGather", ALU.bypass,
                replica_groups=[[0, 2, 4, 6], [1, 3, 5, 7]],
                ins=[tsh_d[:]], outs=[thalf_d[:]],
            )
            for w in tsh_writes:
                add_dep_helper(w, ag, sync=True, reason="tsh->allgather")

            # ---- pre-zero partial buffer
            zt = cpool.tile([128, 8 * (D + 1)], F32)
            nc.vector.memset(zt[:], 0.0)
            zdmas = []
            ZR = 128 * 8
            for r0 in range(0, PBUF, ZR):
                k = min(ZR, PBUF - r0) // 128
                zd = nc.scalar.dma_start(
                    part_d[r0:r0 + k * 128, :].rearrange("(p a) w -> p (a w)", p=128),
                    zt[:, 0:k * (D + 1)])
                zdmas.append(zd)

            # ---- stage 2: strips
            breg = nc.sync.alloc_register("strip_base")
            bases_t = cpool.tile([1, NSTRIP], I32)
            nc.sync.dma_start(bases_t[:], bases_d[:])
            idxt = cpool.tile([128, NSLOT // 16], I16)
            nc.sync.dma_start(idxt[:], idxw_d[:])
            dstmt = cpool.tile([128, NCHTOT], F32)
            nc.sync.dma_start(dstmt[:], dstm_d[:])

            prev_write = None
            IW = SSLOT // 16
            for k in range(NSTRIP):
                xk = gpool.tile([128, NCH, TSTRIDE], F32, tag="xk")
                g = nc.gpsimd.dma_gather(
                    out_ap=xk[:],
                    in_ap=thalf_d[:, 0:TSTRIDE],
                    idxs_ap=idxt[:, k * IW:(k + 1) * IW],
                    num_idxs=SSLOT, num_idxs_reg=SSLOT,
                    elem_size=TSTRIDE, elem_step=TSTRIDE,
                    queue_num=k % dma_queues)
                add_dep_helper(ag, g, sync=True, reason="allgather->gather")
                stk = stpool.tile([128, NCH, 128], F32, tag="stk")
                nc.vector.tensor_tensor(
                    out=stk[:],
                    in0=bcast_mid(dstmt[:, k * NCH:(k + 1) * NCH], 128),
                    in1=tile_mid(iota2[:], NCH),
                    op=ALU.is_equal)
                psk = pspool.tile([128, D + 1], F32, tag="psk", space="PSUM")
                for j in range(NCH):
                    nc.tensor.matmul(psk[:], lhsT=stk[:, j, :],
                                     rhs=xk[:, j, 0:D + 1],
                                     start=(j == 0), stop=(j == NCH - 1))
                ok = okpool.tile([128, D + 1], F32, tag="ok")
                nc.vector.tensor_copy(ok[:], psk[:])
                nc.sync.reg_load(breg, bases_t[0:1, k:k + 1])
                off = nc.sync.snap(breg)
                w = nc.sync.dma_start(part_d[bass.ds(off, 128), :], ok[:])
                for zd in zdmas:
                    add_dep_helper(zd, w, sync=True, reason="zero->strip")
                if prev_write is not None:
                    add_dep_helper(prev_write, w, sync=False, reason="strip order")
                prev_write = w

            # ---- pairwise reduce
            rs = nc.gpsimd.collective_compute(
                "ReduceScatter", ALU.add,
                replica_groups=[[0, 1], [2, 3], [4, 5], [6, 7]],
                ins=[part_d[:]], outs=[rsout_d[:]],
            )
            add_dep_helper(prev_write, rs, sync=True, reason="strips->rs")

            # ---- finalize
            wnodet = cpool.tile([128, D], F32)
            nc.sync.dma_start(wnodet[:], wnode_d[:])
            bnodet = cpool.tile([1, D], F32)
            nc.sync.dma_start(bnodet[:], bnode_d[:])
            bneight = cpool.tile([1, D], F32)
            nc.sync.dma_start(bneight[:], bneigh_d[:])

            for gidx in range(FIN // 128):
                r0 = gidx * 128
                pk = fpool.tile([128, D + 1], F32, tag="pk")
                rd = nc.sync.dma_start(pk[:], rsout_d[r0:r0 + 128, :])
                add_dep_helper(rs, rd, sync=True, reason="rs->finalize")
                hfk = fpool.tile([128, 128], F32, tag="hfk")
                nc.sync.dma_start(hfk[:], hfin_d[r0:r0 + 128, :])
                pstf = ps2pool.tile([128, 128], F32, tag="tr", space="PSUM")
                nc.tensor.transpose(out=pstf[:], in_=hfk[:], identity=ident[:])
                hfT = fpool.tile([128, 128], F32, tag="hfT")
                nc.vector.tensor_copy(hfT[:], pstf[:])
                psn = pspool.tile([128, D], F32, tag="psn", space="PSUM")
                nc.tensor.matmul(psn[:], lhsT=hfT[:], rhs=wnodet[:],
                                 start=True, stop=True)
                hn = fpool.tile([128, D], F32, tag="hn")
                nc.vector.tensor_tensor(out=hn[:], in0=psn[:],
                                        in1=bnodet[:].partition_broadcast(128),
                                        op=ALU.add)
                dn = fpool.tile([128, 1], F32, tag="dn")
                nc.vector.tensor_scalar(out=dn[:], in0=pk[:, D:D + 1],
                                        scalar1=EPS, scalar2=None, op0=ALU.add)
                rcp = fpool.tile([128, 1], F32, tag="rcp")
                nc.vector.reciprocal(rcp[:], dn[:])
                aggs = fpool.tile([128, D], F32, tag="aggs")
                nc.vector.tensor_scalar(out=aggs[:], in0=pk[:, 0:D],
                                        scalar1=rcp[:], scalar2=None,
                                        op0=ALU.mult)
                aggb = fpool.tile([128, D], F32, tag="aggb")
                nc.vector.tensor_tensor(out=aggb[:], in0=aggs[:],
                                        in1=bneight[:].partition_broadcast(128),
                                        op=ALU.add)
                tmp = fpool.tile([128, D], F32, tag="tmp")
                sq1 = fpool.tile([128, 1], F32, tag="sq1")
                nc.vector.tensor_tensor_reduce(
                    out=tmp[:], in0=hn[:], in1=hn[:], scale=1.0, scalar=0.0,
                    op0=ALU.mult, op1=ALU.add, accum_out=sq1[:])
                tmp2 = fpool.tile([128, D], F32, tag="tmp2")
                sq2 = fpool.tile([128, 1], F32, tag="sq2")
                nc.vector.tensor_tensor_reduce(
                    out=tmp2[:], in0=aggb[:], in1=aggb[:], scale=1.0,
                    scalar=sq1[:], op0=ALU.mult, op1=ALU.add, accum_out=sq2[:])
                sqc = fpool.tile([128, 1], F32, tag="sqc")
                nc.vector.tensor_scalar(out=sqc[:], in0=sq2[:], scalar1=EPS,
                                        scalar2=None, op0=ALU.max)
                rsq = fpool.tile([128, 1], F32, tag="rsq")
                nc.scalar.activation(rsq[:], sqc[:], AF.Rsqrt)
                for _ in range(newton):
                    t1 = fpool.tile([128, 1], F32, tag="t1")
                    nc.vector.tensor_tensor(out=t1[:], in0=rsq[:], in1=rsq[:],
                                            op=ALU.mult)
                    nc.vector.tensor_tensor(out=t1[:], in0=t1[:], in1=sqc[:],
                                            op=ALU.mult)
                    nc.vector.tensor_scalar(out=t1[:], in0=t1[:], scalar1=-0.5,
                                            scalar2=1.5, op0=ALU.mult,
                                            op1=ALU.add)
                    rsq2 = fpool.tile([128, 1], F32, tag="rsq")
                    nc.vector.tensor_tensor(out=rsq2[:], in0=rsq[:], in1=t1[:],
                                            op=ALU.mult)
                    rsq = rsq2
                outk = fpool.tile([128, 2 * D], F32, tag="outk")
                nc.vector.tensor_scalar(out=outk[:, 0:D], in0=hn[:],
                                        scalar1=rsq[:], scalar2=None,
                                        op0=ALU.mult)
                nc.vector.tensor_scalar(out=outk[:, D:2 * D], in0=aggb[:],
                                        scalar1=rsq[:], scalar2=None,
                                        op0=ALU.mult)
                nc.sync.dma_start(out_d[r0:r0 + 128, :], outk[:])

    nc.compile()
    return nc


# ---------------------------------------------------------------- entry point
_CACHE = {}


def kernel(**inputs):
    """Full-input GNN attention layer on 8 TRN2 NeuronCores.

    Takes the unsharded inputs of reference.setup_inputs(), distributes
    internally (dst-quarter x src-half edge sharding), returns [N, 256] f32.
    """
    from concourse.bass_utils import run_bass_kernel_spmd

    h = np.ascontiguousarray(np.asarray(inputs["h"], dtype=np.float32))
    src = np.asarray(inputs["src"]).astype(np.int64)
    dst = np.asarray(inputs["dst"]).astype(np.int64)
    N = h.shape[0]
    cfg, idx_all, dstm_all, base_all = prep(src, dst, N)
    maps = host_inputs(
        cfg, h,
        np.ascontiguousarray(np.asarray(inputs["W_coef"], dtype=np.float32)),
        np.ascontiguousarray(np.asarray(inputs["W_red"], dtype=np.float32)),
        np.ascontiguousarray(np.asarray(inputs["W_node"], dtype=np.float32)),
        np.asarray(inputs["b_node"], dtype=np.float32),
        np.ascontiguousarray(np.asarray(inputs["W_neigh"], dtype=np.float32)),
        np.asarray(inputs["b_neigh"], dtype=np.float32),
        idx_all, dstm_all, base_all)
    key = (N, cfg["SSLOT"], cfg["NSTRIP"])
    if key not in _CACHE:
        _CACHE[key] = build(cfg)
    nc = _CACHE[key]
    res = run_bass_kernel_spmd(nc, maps, core_ids=list(range(8)))
    return assemble(cfg, res.results).astype(np.float32)
